# revision 109
# baseline (speedup 1.0000x reference)
"""Trainium2 Bass kernel for AttnBlock++ (GroupNorm + 1x1-conv QKV + dense
attention over 64x64 tokens + 1x1-conv out-proj + residual).

Problem shapes: x [4, 128, 64, 64] f32, four 128x128 NIN weights, GroupNorm(32).

Sharding (8 cores): data-parallel over batch B=4 x query-halves. Core c handles
batch b = c//2 and queries [qh*2048, (qh+1)*2048) with qh = c%2. GroupNorm and
the K/V projections for the batch are recomputed on both cores sharing the
batch (cheap); attention is computed only for the core's query half.

v2 design (cost-model driven; the TimelineSim model is the scoreboard):
 - GroupNorm is folded into the NIN weights on device: W' = a (.) W rows,
   b' = W^T bneg + b, so all NINs consume bf16(x) directly and the stats
   chain only gates the tiny weight-fold ops, not the data path.  The K
   bias is dropped outright (it only shifts scores per-query, which softmax
   cancels).
 - W3 (out-proj) is folded into V host-side: W23 = W2 @ W3, so attention
   produces the final projected values directly and the out-proj matmuls plus
   the post-attention NIN disappear. Per-channel constants (W23^T bneg + b23 +
   b3) ride the residual add in the tail.
 - Scores run as fp8e4 DoubleRow matmuls (0.5 cyc/row): Q/K are quantized to
   fp8 (scaled 2^6 each to clear the subnormal range; compensated in the exp
   scale) and re-laid out into channel-pair form [64, 2, N] via two SBUF->SBUF
   DMAs each (DMA engines are otherwise idle). Score noise from fp8 operands
   is ~4% of the score *std* (relative errors stay relative) -- negligible
   after softmax averaging over 4096 keys and the ~50x residual damping.
 - exp over the 8.4M scores is the dominant elementwise cost; it alternates
   between ACT (real exp -> fp8) and DVE (Schraudolph-style bit-hack exp:
   n = trunc(A*s + B) written as int8 and bitcast to fp8e4; RMS error 2.7%
   vs 2.7% for true-exp-to-fp8 -- the fp8 output quantization dominates
   both).  Pool cannot read PSUM (walrus-enforced), so it gets the SBUF-side
   work instead: the residual adds.
 - x ships twice, split by use: a bf16 copy feeds the GN stats and all NINs
   directly (half the critical DMA bytes, no on-device conversion; bf16
   quantization adds ~5e-6 relative var noise to the stats), and an f32 copy
   of only this core's query half feeds the late residual adds.
 - The in-order PE stream is software-pipelined: PV/sum matmuls are emitted
   OPT_PAIR_LAG chunk-pairs behind the score matmuls (carried across group
   boundaries), so the PE never waits on an exp that was only just issued
   and the two exp engines run concurrently.  Score tiles triple-buffer in
   PSUM; po/ps accumulators live in separate pools (a shared tile creates
   false WAR coupling that costs ~3us).
 - Softmax denominators ride an accumulating DoubleRow matmul with a constant
   32.0 stationary (the 32 cancels against the 2^5 scale folded into W23,
   keeping V' comfortably in fp8 normal range).
 - exp() skips the max-subtraction: scores have std ~0.05, and softmax is
   invariant up to float rounding.
 - GroupNorm stats: bn_stats/bn_aggr on DVE; cross-partition group reduce and
   broadcast via two tiny matmuls; rstd via exp(-0.5*ln(var+eps)) so the
   whole kernel needs a single ACT table set (natural_log_exp).
 - Walrus's TensorScalar encoding has a single sync-wait slot: scalar AP
   operands are always DVE-produced, and TensorScalar ops with a PSUM input
   live on DVE (or use immediate scalars) so no op needs two cross-engine
   waits.
"""

import math

import numpy as np
import ml_dtypes

import concourse.bass as bass
import concourse.tile as tile
from concourse import bacc, mybir
from concourse.bass_utils import run_bass_kernel_spmd

C = 128          # channels
HW = 64
N = HW * HW      # 4096 tokens per batch
B = 4
NCORES = 8
QH = N // 2      # queries per core
NGROUPS = 32
GS = C // NGROUPS  # channels per group
EPS = 1e-6
FD = 512         # NIN moving free-dim tile
NCH = N // 128   # key chunks (32)
BN_SUB = 512     # bn_stats free-dim limit

F32 = mybir.dt.float32
BF16 = mybir.dt.bfloat16
FP8 = mybir.dt.float8e4
I8 = mybir.dt.int8
AF = mybir.ActivationFunctionType
ALU = mybir.AluOpType
DROW = mybir.MatmulPerfMode.DoubleRow

# quantization scales
QK_SHIFT = 6          # Q and K each scaled by 2^6 before fp8 quantization
S_SCALE = 2.0 ** (-2 * QK_SHIFT)   # undo in exp argument
V_SCALE = 32.0        # W23 scaled on device by a*32; ones stationary = 32.0
A_HACK = (8.0 / math.log(2.0)) * S_SCALE
B_HACK = 56.32        # 7*8 + calibrated offset for truncating conversion

# cpack columns: b0s | b1 | b233 | gnsc | gnbi | eps | pad | pad
NCONST = 8

# exp engine assignment per chunk-pair index (16 per group):
# A=ACT real exp, D=DVE hack.  Pool (gpsimd) cannot read PSUM so it gets the
# SBUF-side elementwise work (final residual adds) instead.  Each group leads
# with AA so the DVE stream reaches the previous group's normalize chain
# (recip/AT) immediately, releasing the accumulator WAR for this group's
# first PV.
EXP_PATTERNS = ["ADAADADAADADADAD", "ADAADADAADADADAD"]

# scheduling knobs (tuned via TimelineSim sweeps)
OPT_AFD = 512               # attention query-group width
OPT_NIN_INTERLEAVE = False  # spread late K/V' conversions into early attention
OPT_PAIR_LAG = 3            # emission lag of PV/sum behind scores (chunk-pairs)
OPT_XB_POOL = True          # residual-bias add on Pool vs DVE
OPT_X_DMAS = 4              # number of DMAs for the (bf16) x load
OPT_PSC_BUFS = 3            # score-buffer count
OPT_ACC_BUFS = 1            # po/ps accumulator buffering
OPT_VBLOCK = 8              # key-chunks per V' psum block (4 or 8)
OPT_TAIL_HALVES = False     # with the hoisted recip, one full-width tail wins
OPT_GROUP_CARRY = True      # carry PV lag across group boundaries
OPT_ACC_SPLIT = True        # po/ps in separate pools vs one [C,2,AFD] tile
OPT_PSC_SPLIT = None        # None, or (bufsA, bufsD): per-engine psc pools
OPT_PS_FIRST = True         # emit the last ps before the last po per group
OPT_LAST_YS_DVE = True      # final group's residual add on DVE (drain shave)
OPT_DIV_TAIL = False        # ACT copies ps to SBUF, DVE divides (1 op vs 2)
OPT_FULL_RECIP = True       # one full-width recip per group vs per-half


def _build_program(loop_reps=None):
    nc = bacc.Bacc("TRN2", target_bir_lowering=False, debug=False,
                   num_devices=NCORES)

    def din(name, shape, dt=F32):
        return nc.dram_tensor(name, shape, dt, kind="ExternalInput").ap()

    # xf: full batch image, channels-major, with the column-halves swapped
    # host-side for odd cores so THIS core's 2048 query columns are always
    # xf[:, :QH]. Key order only permutes the softmax sum.
    xbf = din("xbf", [C, N], BF16)           # bf16 x: stats + NIN inputs
    xq = din("xq", [C, QH])                  # f32 x, query half: residual
    wbf = din("wbf", [C, 3 * C], BF16)       # W0*scale*2^6 | W1*2^6 | W23
    cpack = din("cpack", [C, NCONST])
    gmat = din("gmat", [C, C])               # group-averaging matrix
    y = nc.dram_tensor("y", [C, QH], F32, kind="ExternalOutput").ap()

    import contextlib

    with tile.TileContext(nc) as tc:
        loop_cm = (tc.For_i(0, loop_reps, 1) if loop_reps
                   else contextlib.nullcontext())
        with (
            loop_cm,
            tc.tile_pool(name="const", bufs=1) as constp,
            tc.tile_pool(name="data", bufs=1) as datap,
            tc.tile_pool(name="small", bufs=1) as smallp,
            tc.tile_pool(name="pexp", bufs=8) as ppool,
            tc.tile_pool(name="work", bufs=3) as workp,
            tc.tile_pool(name="mm", bufs=OPT_PSC_BUFS, space="PSUM") as mmp,
            tc.tile_pool(name="accs", bufs=OPT_ACC_BUFS, space="PSUM") as accsp,
        ):
            if OPT_PSC_SPLIT:
                TAGKW = dict(tag="mmA", bufs=OPT_PSC_SPLIT[0])
            else:
                TAGKW = dict(tag="mm")

            # ---- load x first (the GN stats chain gates everything).
            # x is shipped twice, split by use: a bf16 copy (H) feeds the
            # stats and every NIN directly (half the critical DMA bytes, no
            # on-device conversion), and an f32 copy of only this core's
            # query half feeds the residual add much later.
            H = datap.tile([C, N], BF16, tag="h")
            nx = OPT_X_DMAS
            for j in range(nx):
                js = slice(j * (N // nx), (j + 1) * (N // nx))
                nc.sync.dma_start(out=H[:, js], in_=xbf[:, js])
            XQ = datap.tile([C, QH], F32, tag="xq")

            # ---- constants -------------------------------------------------
            WB = constp.tile([C, 3 * C], BF16, tag="wb")
            nc.sync.dma_start(out=WB, in_=wbf)
            CP = constp.tile([C, NCONST], F32, tag="cp")
            nc.sync.dma_start(out=CP, in_=cpack)
            gm = constp.tile([C, C], F32, tag="gm")
            nc.sync.dma_start(out=gm, in_=gmat)
            # residual input: needed only by the attention tails
            for j in range(4):
                js = slice(j * (QH // 4), (j + 1) * (QH // 4))
                nc.sync.dma_start(out=XQ[:, js], in_=xq[:, js])

            # re-slice constants through DVE (single-wait rule)
            b0s = constp.tile([C, 1], F32, tag="b0s")
            nc.vector.tensor_copy(b0s, CP[:, 0:1])
            b233 = constp.tile([C, 1], F32, tag="b233")
            nc.vector.tensor_copy(b233, CP[:, 2:3])
            gnsct = constp.tile([C, 1], F32, tag="gnsc")
            nc.vector.tensor_copy(gnsct, CP[:, 3:4])
            gnbit = constp.tile([C, 1], F32, tag="gnbi")
            nc.vector.tensor_copy(gnbit, CP[:, 4:5])
            epst = constp.tile([C, 1], F32, tag="eps")
            nc.vector.tensor_copy(epst, CP[:, 5:6])
            b0shi = constp.tile([64, 1], F32, tag="b0shi")
            nc.vector.tensor_copy(b0shi, CP[0:64, 6:7])
            ones32 = constp.tile([C, 2, C], FP8, tag="ones32")
            nc.gpsimd.memset(ones32, V_SCALE)

            # ---- GroupNorm stats ------------------------------------------
            stats = smallp.tile([C, N // BN_SUB, 6], F32, tag="bnstats")
            for j in range(N // BN_SUB):
                nc.vector.bn_stats(out=stats[:, j, :],
                                   in_=H[:, j * BN_SUB:(j + 1) * BN_SUB])
            mv = smallp.tile([C, 2], F32, tag="mv")
            nc.vector.bn_aggr(out=mv, in_=stats)
            # st = [mean, E[x^2]] per partition
            st = smallp.tile([C, 2], F32, tag="st")
            nc.vector.tensor_copy(st[:, 0:1], mv[:, 0:1])
            nc.vector.tensor_tensor(st[:, 1:2], mv[:, 0:1], mv[:, 0:1],
                                    ALU.mult)
            nc.vector.tensor_tensor(st[:, 1:2], st[:, 1:2], mv[:, 1:2],
                                    ALU.add)
            # cross-partition group reduce + broadcast in ONE matmul: the
            # host pre-multiplies the indicator matrices into a symmetric
            # [C,C] group-averaging matrix (saves a PE round-trip on the
            # startup-critical stats chain)
            pb = mmp.tile([C, 2], F32, **TAGKW)
            nc.tensor.matmul(pb, lhsT=gm, rhs=st, start=True, stop=True)
            gmean = smallp.tile([C, 1], F32, tag="gmean")
            nc.vector.tensor_copy(gmean, pb[:, 0:1])
            varg = smallp.tile([C, 1], F32, tag="varg")
            nc.vector.tensor_tensor(varg, gmean, gmean, ALU.mult)
            nc.vector.tensor_tensor(varg, pb[:, 1:2], varg, ALU.subtract)
            # rstd = (var+eps)^-0.5 via exp(-0.5*ln(var+eps)); Ln and Exp
            # share one ACT table set
            lnv = smallp.tile([C, 1], F32, tag="lnv")
            nc.scalar.activation(out=lnv, in_=varg, func=AF.Ln, bias=epst,
                                 scale=1.0)
            rstd = smallp.tile([C, 1], F32, tag="rstd")
            nc.scalar.activation(out=rstd, in_=lnv, func=AF.Exp, scale=-0.5)
            a_t = smallp.tile([C, 1], F32, tag="a_t")
            nc.vector.tensor_tensor(a_t, rstd, gnsct, ALU.mult)
            a32_t = smallp.tile([C, 1], F32, tag="a32_t")
            nc.vector.tensor_scalar_mul(a32_t, a_t, V_SCALE)
            bneg = smallp.tile([C, 1], F32, tag="bneg")
            nc.vector.tensor_tensor(bneg, gmean, a_t, ALU.mult)
            nc.vector.tensor_tensor(bneg, gnbit, bneg, ALU.subtract)

            # ---- fold GN affine into the weights --------------------------
            W0h = constp.tile([C, C], BF16, tag="w0h")
            nc.vector.tensor_scalar_mul(W0h, WB[:, 0:C], a_t)
            W1h = constp.tile([C, C], BF16, tag="w1h")
            nc.vector.tensor_scalar_mul(W1h, WB[:, C:2 * C], a_t)
            W23h = constp.tile([C, C], BF16, tag="w23h")
            nc.vector.tensor_scalar_mul(W23h, WB[:, 2 * C:3 * C], a32_t)
            # bias matvecs: b' = W^T bneg (+ packed consts).  The K bias is
            # dropped entirely: it only contributes a per-query score shift,
            # which softmax cancels.
            bnegb = smallp.tile([C, 1], BF16, tag="bnegb")
            nc.vector.tensor_copy(bnegb, bneg)
            pbias = mmp.tile([C, 3], F32, **TAGKW)
            nc.tensor.matmul(pbias[:, 0:1], lhsT=WB[:, 0:C], rhs=bnegb,
                             start=True, stop=True)
            nc.tensor.matmul(pbias[:, 1:2], lhsT=WB[:, 2 * C:3 * C],
                             rhs=bnegb, start=True, stop=True)
            # high-half Q bias delivered straight to partitions 0-63 (for the
            # chunk-0 fast path's pair-layout conversion): matvec over W0's
            # high output columns, plus the host-packed b0s high half
            nc.tensor.matmul(pbias[0:64, 2:3], lhsT=WB[:, 64:128],
                             rhs=bnegb, start=True, stop=True)
            b0d = smallp.tile([C, 1], F32, tag="b0d")
            nc.vector.tensor_tensor(b0d, pbias[:, 0:1], b0s, ALU.add)
            b0dhi = smallp.tile([64, 1], F32, tag="b0dhi")
            nc.vector.tensor_tensor(b0dhi, pbias[0:64, 2:3], b0shi,
                                    ALU.add)
            b233d = smallp.tile([C, 1], F32, tag="b233d")
            nc.vector.tensor_tensor(b233d, pbias[:, 1:2], b233, ALU.add)

            # ---- NIN phase: K, Q, V' --------------------------------------
            Kf = datap.tile([C, N], FP8, tag="kf")
            Qf = datap.tile([C, QH], FP8, tag="qf")
            KH = datap.tile([64, 2, N], FP8, tag="kh")
            QHt = datap.tile([64, 2, QH], FP8, tag="qht")
            # V' transposed, fp8, DoubleRow pairing [m-part, cp, parity, k]
            VT = datap.tile([C, NCH // 2, 2, C], FP8, tag="vt")

            def k_chunk(j):
                js = slice(j * FD, (j + 1) * FD)
                pk = mmp.tile([C, FD], F32, **TAGKW)
                nc.tensor.matmul(pk, lhsT=W1h, rhs=H[:, js],
                                 start=True, stop=True)
                nc.scalar.activation(out=Kf[:, js], in_=pk, func=AF.Copy,
                                     scale=1.0, bias=0.0)

            def q_chunk(j):
                js = slice(j * FD, (j + 1) * FD)
                pq = mmp.tile([C, FD], F32, **TAGKW)
                nc.tensor.matmul(pq, lhsT=W0h, rhs=H[:, js],
                                 start=True, stop=True)
                nc.vector.tensor_scalar_add(out=Qf[:, js], in0=pq,
                                            scalar1=b0d)

            def pair_dma(dst, src, js):
                # batched channel-pair relayout: [128, n] -> [64, 2, n]
                nc.sync.dma_start(out=dst[:, 0, js], in_=src[0:64, js])
                nc.sync.dma_start(out=dst[:, 1, js], in_=src[64:128, js])

            def v_block(t):
                # OPT_VBLOCK key-chunks -> one psum tile -> one ACT copy
                nb = OPT_VBLOCK
                pv = mmp.tile([C, nb, C], F32, **TAGKW)
                for s in range(nb):
                    ch = nb * t + s
                    nc.tensor.matmul(pv[:, s, :],
                                     lhsT=H[:, ch * 128:(ch + 1) * 128],
                                     rhs=W23h, start=True, stop=True)
                nc.scalar.activation(
                    out=VT[:, (nb // 2) * t:(nb // 2) * (t + 1), :, :],
                    in_=pv, func=AF.Copy, scale=1.0, bias=0.0)

            # Pre-attention NIN: only what the first attention pairs need.
            # The second half of K, and V' blocks 1-3, are emitted interleaved
            # into the early attention stream (their ACT-side conversions then
            # fill gaps in the exp window instead of delaying its start).
            for j in range(4):
                k_chunk(j)
                q_chunk(j)
                if j == 0:
                    # first chunk of keys and queries ASAP
                    pair_dma(KH, Kf, slice(0, FD))
                    pair_dma(QHt, Qf, slice(0, FD))
            pair_dma(KH, Kf, slice(FD, 4 * FD))
            pair_dma(QHt, Qf, slice(FD, 4 * FD))
            nvb = NCH // OPT_VBLOCK      # number of V' blocks (8 or 4)
            v_block(0)
            if OPT_VBLOCK == 4:
                v_block(1)

            if not OPT_NIN_INTERLEAVE:
                for j in range(4, 8):
                    k_chunk(j)
                pair_dma(KH, Kf, slice(4 * FD, 8 * FD))
                for t in range(2 if OPT_VBLOCK == 4 else 1, nvb):
                    v_block(t)

            # emitted between early attention pairs (global pair index):
            # each K/V' conversion lands on ACT just before its consumer
            # needs it, instead of delaying the start of the exp window
            if OPT_VBLOCK == 4:
                NIN_REST = {
                    2: lambda: v_block(2),
                    3: lambda: k_chunk(4),
                    4: lambda: v_block(3),
                    5: lambda: (k_chunk(5),
                                pair_dma(KH, Kf, slice(4 * FD, 6 * FD))),
                    6: lambda: v_block(4),
                    7: lambda: k_chunk(6),
                    8: lambda: v_block(5),
                    9: lambda: (k_chunk(7),
                                pair_dma(KH, Kf, slice(6 * FD, 8 * FD))),
                    10: lambda: v_block(6),
                    12: lambda: v_block(7),
                }
            else:
                NIN_REST = {
                    2: lambda: v_block(1),
                    3: lambda: k_chunk(4),
                    5: lambda: (k_chunk(5),
                                pair_dma(KH, Kf, slice(4 * FD, 6 * FD))),
                    6: lambda: v_block(2),
                    7: lambda: k_chunk(6),
                    9: lambda: (k_chunk(7),
                                pair_dma(KH, Kf, slice(6 * FD, 8 * FD))),
                    10: lambda: v_block(3),
                }

            def nin_rest(step):
                if OPT_NIN_INTERLEAVE and step in NIN_REST:
                    NIN_REST[step]()

            # ---- attention -------------------------------------------------
            # Two levels of software pipelining: within a group the po/ps
            # (PV + sum) matmuls are emitted PAIR_LAG chunk-pairs behind the
            # score matmuls, so the in-order PE stream never waits on an exp
            # that was only just issued -- ACT and DVE exps run concurrently
            # behind the PE.  Across groups the normalize/residual tail is
            # emitted one group late (as in v1).
            PAIR_LAG = OPT_PAIR_LAG
            npair = NCH // 2
            AFD = OPT_AFD
            NAG = QH // AFD

            def attn_tail(g, po, ps, XB):
                nh = 2 if OPT_TAIL_HALVES else 1
                HF = AFD // nh
                RF = None
                if OPT_FULL_RECIP:
                    RF = workp.tile([C, AFD], F32, tag="rf")
                    nc.vector.reciprocal_approx_fast(out=RF, in_=ps)
                for h in range(nh):
                    qs = slice(g * AFD + h * HF, g * AFD + (h + 1) * HF)
                    hs = slice(h * HF, (h + 1) * HF)
                    AT = workp.tile([C, HF], F32, tag="at")
                    if OPT_FULL_RECIP:
                        nc.vector.tensor_tensor(AT, po[:, hs], RF[:, hs],
                                                ALU.mult)
                    elif OPT_DIV_TAIL:
                        # ACT (idle) stages ps to SBUF; one DVE divide
                        # replaces the recip+multiply pair
                        PS = workp.tile([C, HF], F32, tag="pss")
                        nc.scalar.activation(out=PS, in_=ps[:, hs],
                                             func=AF.Copy, scale=1.0,
                                             bias=0.0)
                        nc.vector.tensor_tensor(AT, po[:, hs], PS,
                                                ALU.divide)
                    else:
                        R = workp.tile([C, HF], F32, tag="r")
                        nc.vector.reciprocal_approx_fast(out=R, in_=ps[:, hs])
                        nc.vector.tensor_tensor(AT, po[:, hs], R, ALU.mult)
                    YS = workp.tile([C, HF], F32, tag="ys")
                    if OPT_LAST_YS_DVE and g == NAG - 1 and h == nh - 1:
                        nc.vector.tensor_tensor(YS, AT, XB[:, hs], ALU.add)
                    else:
                        nc.gpsimd.tensor_tensor(YS, AT, XB[:, hs], ALU.add)
                    nc.sync.dma_start(out=y[:, qs], in_=YS)

            pend_p = []

            def emit_pv():
                g, cp, po, ps, P, XB = pend_p.pop(0)
                last = cp == npair - 1
                if last and OPT_PS_FIRST:
                    # finishing ps before po lets the recip start under the
                    # final PV matmul
                    nc.tensor.matmul(ps, lhsT=ones32, rhs=P,
                                     start=False, stop=True, perf_mode=DROW)
                    nc.tensor.matmul(po, lhsT=VT[:, cp, :, :], rhs=P,
                                     start=False, stop=True, perf_mode=DROW)
                else:
                    nc.tensor.matmul(po, lhsT=VT[:, cp, :, :], rhs=P,
                                     start=(cp == 0), stop=last,
                                     perf_mode=DROW)
                    nc.tensor.matmul(ps, lhsT=ones32, rhs=P,
                                     start=(cp == 0), stop=last,
                                     perf_mode=DROW)
                if last:
                    attn_tail(g, po, ps, XB)

            for g in range(NAG):
                qs = slice(g * AFD, (g + 1) * AFD)
                if OPT_ACC_SPLIT:
                    po = accsp.tile([C, AFD], F32, tag="po")
                    ps = accsp.tile([C, AFD], F32, tag="ps")
                else:
                    acc = accsp.tile([C, 2, AFD], F32, tag="acc")
                    po = acc[:, 0, :]
                    ps = acc[:, 1, :]
                # residual + bias precomputed on Pool (off the tail chain)
                XB = workp.tile([C, AFD], F32, tag="xb")
                if OPT_XB_POOL:
                    nc.gpsimd.tensor_scalar(out=XB, in0=XQ[:, qs],
                                            scalar1=b233d, scalar2=None,
                                            op0=ALU.add)
                else:
                    nc.vector.tensor_scalar_add(out=XB, in0=XQ[:, qs],
                                                scalar1=b233d)
                pattern = EXP_PATTERNS[g % len(EXP_PATTERNS)]
                for cp in range(npair):
                    if OPT_PSC_SPLIT:
                        ba, bd = OPT_PSC_SPLIT
                        if pattern[cp] == "A":
                            psc = mmp.tile([C, 2, AFD], F32, tag="mmA",
                                           bufs=ba)
                        else:
                            psc = mmp.tile([C, 2, AFD], F32, tag="mmD",
                                           bufs=bd)
                    else:
                        psc = mmp.tile([C, 2, AFD], F32, tag="mm")
                    for i in range(2):
                        ch = 2 * cp + i
                        nc.tensor.matmul(
                            psc[:, i, :],
                            lhsT=KH[:, :, ch * 128:(ch + 1) * 128],
                            rhs=QHt[:, :, qs],
                            start=True, stop=True, perf_mode=DROW)
                    P = ppool.tile([C, 2, AFD], FP8, tag="p")
                    eng = pattern[cp]
                    if eng == "A":
                        nc.scalar.activation(out=P, in_=psc, func=AF.Exp,
                                             scale=S_SCALE)
                    else:
                        nc.vector.tensor_scalar(
                            out=P.bitcast(I8), in0=psc, scalar1=A_HACK,
                            scalar2=B_HACK, op0=ALU.mult, op1=ALU.add)
                    pend_p.append((g, cp, po, ps, P, XB))
                    if len(pend_p) > PAIR_LAG:
                        emit_pv()
                    if g == 0:
                        nin_rest(cp)
                if not OPT_GROUP_CARRY:
                    while pend_p:
                        emit_pv()
            while pend_p:
                emit_pv()



    nc.compile()
    return nc


_PROGRAM = None


def _get_program():
    global _PROGRAM
    if _PROGRAM is None:
        _PROGRAM = _build_program()
    return _PROGRAM


_RUNNER = None


def _get_runner():
    """Build (once) a cached jitted multi-core executor for the program."""
    global _RUNNER
    if _RUNNER is not None:
        return _RUNNER
    import jax
    from concourse import bass2jax, mybir as _mb

    nc = _get_program()
    bass2jax.install_neuronx_cc_hook()
    assert nc.dbg_addr is None
    partition_name = (nc.partition_id_tensor.name
                      if nc.partition_id_tensor else None)
    in_names, out_names, out_avals = [], [], []
    for alloc in nc.m.functions[0].allocations:
        if not isinstance(alloc, _mb.MemoryLocationSet):
            continue
        name = alloc.memorylocations[0].name
        if alloc.kind == "ExternalInput":
            if name != partition_name:
                in_names.append(name)
        elif alloc.kind == "ExternalOutput":
            shape = tuple(alloc.tensor_shape)
            dtype = _mb.dt.np(alloc.dtype)
            out_avals.append(jax.core.ShapedArray(shape, dtype))
    n_params = len(in_names)
    n_outs = len(out_avals)
    out_names = [a.memorylocations[0].name
                 for a in nc.m.functions[0].allocations
                 if isinstance(a, _mb.MemoryLocationSet)
                 and a.kind == "ExternalOutput"]
    all_names = list(in_names) + list(out_names)
    if partition_name is not None:
        all_names.append(partition_name)

    def _body(*args):
        operands = list(args)
        if partition_name is not None:
            operands.append(bass2jax.partition_id_tensor())
        outs = bass2jax._bass_exec_p.bind(
            *operands,
            out_avals=tuple(out_avals),
            in_names=tuple(all_names),
            out_names=tuple(out_names),
            lowering_input_output_aliases=(),
            sim_require_finite=True,
            sim_require_nnan=True,
            nc=nc,
        )
        return tuple(outs)

    devices = jax.devices()[:NCORES]
    mesh = bass2jax.Mesh(np.asarray(devices), ("core",))
    in_specs = (bass2jax.PartitionSpec("core"),) * (n_params + n_outs)
    out_specs = (bass2jax.PartitionSpec("core"),) * n_outs
    donate = tuple(range(n_params, n_params + n_outs))
    sharded = jax.jit(
        bass2jax.shard_map(_body, mesh=mesh, in_specs=in_specs,
                           out_specs=out_specs, check_rep=False),
        donate_argnums=donate, keep_unused=True,
    )
    _RUNNER = (sharded, in_names, out_names, out_avals)
    return _RUNNER


def _run_cached(in_maps):
    sharded, in_names, out_names, out_avals = _get_runner()
    concat_in = [
        np.concatenate([np.asarray(in_maps[c][nm]) for c in range(NCORES)],
                       axis=0)
        for nm in in_names
    ]
    concat_zeros = [
        np.zeros((NCORES * a.shape[0], *a.shape[1:]), a.dtype)
        for a in out_avals
    ]
    out_arrs = sharded(*concat_in, *concat_zeros)
    return [
        {nm: np.asarray(out_arrs[i]).reshape(NCORES, *out_avals[i].shape)[c]
         for i, nm in enumerate(out_names)}
        for c in range(NCORES)
    ]


def _make_in_maps(x, gn_scale, gn_bias, Ws, bs):
    scale = 1.0 / math.sqrt(C)
    qk_mul = float(2.0 ** QK_SHIFT)
    bf = ml_dtypes.bfloat16
    W0 = np.asarray(Ws[0], np.float32) * scale * qk_mul
    W1 = np.asarray(Ws[1], np.float32) * qk_mul
    W23 = (np.asarray(Ws[2], np.float32) @ np.asarray(Ws[3], np.float32))
    wbf = np.concatenate([W0, W1, W23], axis=1).astype(bf)
    b233 = (np.asarray(Ws[3], np.float32).T @ np.asarray(bs[2], np.float32)
            + np.asarray(bs[3], np.float32))
    cpack = np.zeros((C, NCONST), np.float32)
    cpack[:, 0] = np.asarray(bs[0], np.float32) * scale * qk_mul
    cpack[:, 1] = np.asarray(bs[1], np.float32) * qk_mul
    cpack[:, 2] = b233
    cpack[:, 3] = np.asarray(gn_scale, np.float32)
    cpack[:, 4] = np.asarray(gn_bias, np.float32)
    cpack[:, 5] = EPS
    cpack[0:64, 6] = (np.asarray(bs[0], np.float32) * scale * qk_mul)[64:128]
    gind = np.zeros((C, NGROUPS), np.float32)
    for c in range(C):
        gind[c, c // GS] = 1.0
    gmat = (gind @ gind.T) / GS

    xr = np.asarray(x, np.float32).reshape(B, C, N)
    in_maps = []
    for core in range(NCORES):
        b, qh = core // 2, core % 2
        xfb = xr[b] if qh == 0 else np.concatenate(
            [xr[b][:, QH:], xr[b][:, :QH]], axis=1)
        in_maps.append({
            "xbf": np.ascontiguousarray(xfb).astype(bf),
            "xq": np.ascontiguousarray(xfb[:, :QH]),
            "wbf": wbf,
            "cpack": cpack,
            "gmat": gmat,
        })
    return in_maps


def _assemble(results):
    y = np.empty((B, C, N), np.float32)
    for core in range(NCORES):
        b, qh = core // 2, core % 2
        y[b][:, qh * QH:(qh + 1) * QH] = results[core]["y"]
    return y.reshape(B, C, HW, HW)


def kernel(x, gn_scale, gn_bias, W0, b0, W1, b1, W2, b2, W3, b3,
           _trace=False, _tmpdir=None):
    in_maps = _make_in_maps(x, gn_scale, gn_bias,
                            [W0, W1, W2, W3], [b0, b1, b2, b3])
    if _trace:
        res = run_bass_kernel_spmd(_get_program(), in_maps,
                                   core_ids=list(range(NCORES)),
                                   trace=True, tmpdir=_tmpdir)
        return _assemble(res.results), res
    return _assemble(_run_cached(in_maps))
